# revision 13
# baseline (speedup 1.0000x reference)
"""GQA kernel for Trainium2, 8 NeuronCores.

Sharding: core c = b*4 + g handles batch b, kv-head g (4 query heads).
Each core computes (all matmuls bf16, fp32 PSUM):
  Q_g^T = Wq_g @ x_q^T   [4 heads][128, S]  (1/sqrt(D) folded into Wq host-side)
  K_g^T = Wk_g @ x_k^T   [128, S]
  V_g   = transpose(Wv_g @ x_v^T)           [S, 128] via PE transpose
  S^T   = K_kt^T Q       [k, q] -> +mask (diag tiles, DVE) -> exp (ACT)
  o^T  += V[kt] @ P~     (PSUM accum), l += ones^T P~
  o_norm^T = o^T * bcast(1/l)  (approx recip + SWDGE partition-broadcast)
  partial = o_norm @ Wo_g^T  [S, E]
Host sums the 4 partials per batch.

Perf structure (vs the naive version):
- All DRAM params are host-tiled to exactly match their SBUF layout, so
  every load is a big contiguous DMA (16KB rows).
- DMAs are split across both HW DGE queues (Sync + Scalar engines);
  out-writes also use the GpSimd SWDGE queue. The naive version pushed
  all 344 DMAs through the single Sync queue (~208us serialized).
- Attention is software-pipelined with a 4-tile score lookahead so the
  PE never waits on the score->exp->AV chain.
- Phases are interleaved (proj c0, att q0, proj c1, outproj q0, ...) so
  projection DMA latency hides under attention compute.
- Softmax normalization: reciprocal_approx_fast on [1,512] + SBUF->SBUF
  partition-broadcast DMA, replacing a DRAM round-trip + 3.3us full-width
  DVE reciprocal per head.
"""

import sys

import numpy as np

for _p in ("/opt/trn_rl_repo",):
    if _p not in sys.path:
        sys.path.insert(0, _p)

import ml_dtypes

import concourse.bass as bass
import concourse.mybir as mybir
from concourse import bacc
from concourse.bass_utils import run_bass_kernel_spmd
from concourse.masks import make_identity
from concourse.tile import TileContext

B, S, E = 2, 2048, 2048
H, HKV = 16, 4
D = E // H  # 128
G = H // HKV  # 4 query heads per kv head
GD = G * D  # 512
NCORES = B * HKV  # 8
SC = 512  # s/q chunk width (free dim of matmuls)
NSC = S // SC  # 4
NET = E // 128  # 16 e-tiles (contraction)
NKT = S // 128  # 16 k-tiles
SCALE = 1.0 / float(np.sqrt(D))

F32 = mybir.dt.float32
BF16 = mybir.dt.bfloat16
AF = mybir.ActivationFunctionType
NPBF = np.dtype(ml_dtypes.bfloat16)


def build_nc():
    nc = bacc.Bacc()
    # x tensors host-tiled: [p, chunk, etile, s2] so chunk loads are
    # contiguous 16KB rows per partition.
    xq = nc.declare_dram_parameter("xq", [128, NSC, NET, SC], BF16, isOutput=False)
    xk = nc.declare_dram_parameter("xk", [128, NSC, NET, SC], BF16, isOutput=False)
    xv = nc.declare_dram_parameter("xv", [128, NSC, NET, SC], BF16, isOutput=False)
    # weights host-tiled to SBUF layout
    wq = nc.declare_dram_parameter("wq", [128, NET, GD], BF16, isOutput=False)
    wk = nc.declare_dram_parameter("wk", [128, NET, D], BF16, isOutput=False)
    wv = nc.declare_dram_parameter("wv", [128, NET, D], BF16, isOutput=False)
    wo = nc.declare_dram_parameter("wo", [128, G, E], BF16, isOutput=False)
    out = nc.declare_dram_parameter("out", [S, E], BF16, isOutput=True)

    with TileContext(nc) as tc:
        with (
            tc.tile_pool(name="singles", bufs=1) as singles,
            tc.tile_pool(name="xsp", bufs=5) as xsp,
            tc.tile_pool(name="pexp", bufs=8) as pexp,
            tc.tile_pool(name="vtp", bufs=2) as vtp,
            tc.tile_pool(name="ob", bufs=8) as obp,
            tc.tile_pool(name="rlp", bufs=4) as rlp,
            tc.tile_pool(name="rbp", bufs=4) as rbp,
            tc.tile_pool(name="acc", bufs=4, space="PSUM") as acc,
            tc.tile_pool(name="ops", bufs=2, space="PSUM") as ops,
            tc.tile_pool(name="lps", bufs=1, space="PSUM") as lps,
            tc.tile_pool(name="trp", bufs=1, space="PSUM") as trp,
        ):
            # ---- SBUF-resident tensors ----
            wq_sb = singles.tile([128, NET, GD], BF16)  # 16KB/p
            wk_sb = singles.tile([128, NET, D], BF16)  # 4KB/p
            wv_sb = singles.tile([128, NET, D], BF16)  # 4KB/p
            wo_sb = singles.tile([128, G, E], BF16)  # 16KB/p
            mask_sb = singles.tile([128, 4, SC], F32)  # 8KB/p
            ident_f = singles.tile([128, 128], F32)
            ident = singles.tile([128, 128], BF16)
            ones_f = singles.tile([128, 1], F32)
            ones = singles.tile([128, 1], BF16)
            qT = singles.tile([128, G, S], BF16)  # 16KB/p
            kT = singles.tile([128, S], BF16)  # 4KB/p
            v_sb = singles.tile([128, NKT, D], BF16)  # 4KB/p
            onrm = singles.tile([128, G, S], BF16)  # 16KB/p

            make_identity(nc, ident_f)
            nc.scalar.activation(out=ident[:], in_=ident_f[:], func=AF.Copy)
            nc.vector.memset(ones_f, 1.0)
            nc.scalar.activation(out=ones[:], in_=ones_f[:], func=AF.Copy)
            # causal additive mask tiles, generated on-device:
            # mask[p, j, q] = 0 if q >= p + 128*j else -1e9
            for j in range(4):
                nc.gpsimd.memset(mask_sb[:, j, :], 0.0)
                nc.gpsimd.affine_select(
                    out=mask_sb[:, j, :],
                    in_=mask_sb[:, j, :],
                    compare_op=mybir.AluOpType.is_ge,
                    fill=-1e9,
                    base=-128 * j,
                    channel_multiplier=-1,
                    pattern=[[1, SC]],
                )

            xts = {}

            def alloc_x(c):
                tiles = []
                for nm in ("q", "k", "v"):
                    xt = xsp.tile([128, NET, SC], BF16, tag="x", name=f"x{nm}")
                    tiles.append(xt)
                xts[c] = tuple(tiles)

            def load_half(c, idx, xx, half):
                # half 0 -> sync queue, tiles 0..7; half 1 -> scalar, 8..15
                xt = xts[c][idx]
                eng = nc.sync if half == 0 else nc.scalar
                sl = slice(0, 8) if half == 0 else slice(8, 16)
                eng.dma_start(out=xt[:, sl, :], in_=xx[:, c, sl, :])

            # ---- startup: chunk-0 x + early weights, quartered for fast
            # first-matmul; sync and scalar HW DGE queues run in parallel ----
            alloc_x(0)
            alloc_x(1)
            xt0q = xts[0][0]
            for j in range(4):
                eng = nc.sync if j % 2 == 0 else nc.scalar
                eng.dma_start(
                    out=xt0q[:, j * 4 : (j + 1) * 4, :],
                    in_=xq[:, 0, j * 4 : (j + 1) * 4, :],
                )
                eng.dma_start(
                    out=wq_sb[:, j * 4 : (j + 1) * 4, :],
                    in_=wq[:, j * 4 : (j + 1) * 4, :],
                )
            nc.gpsimd.dma_start(out=wk_sb[:], in_=wk[:])
            nc.gpsimd.dma_start(out=wv_sb[:], in_=wv[:])
            load_half(0, 1, xk, 0)
            load_half(0, 1, xk, 1)
            xt0v = xts[0][2]
            nc.gpsimd.dma_start(out=xt0v[:, :8, :], in_=xv[:, 0, :8, :])
            nc.gpsimd.dma_start(out=xt0v[:, 8:, :], in_=xv[:, 0, 8:, :])
            load_half(1, 0, xq, 0)
            load_half(1, 0, xq, 1)
            load_half(1, 1, xk, 0)
            load_half(1, 1, xk, 1)

            def load_wo():
                nc.sync.dma_start(out=wo_sb[:, :2, :], in_=wo[:, :2, :])
                nc.scalar.dma_start(out=wo_sb[:, 2:, :], in_=wo[:, 2:, :])

            def proj(c):
                ssl = slice(c * SC, (c + 1) * SC)
                xtq, xtk, xtv = xts[c]
                for h in range(G):
                    ps = acc.tile([128, SC], F32, tag="acc")
                    for t in range(NET):
                        nc.tensor.matmul(
                            ps[:],
                            lhsT=wq_sb[:, t, h * D : (h + 1) * D],
                            rhs=xtq[:, t, :],
                            start=(t == 0),
                            stop=(t == NET - 1),
                        )
                    nc.scalar.activation(out=qT[:, h, ssl], in_=ps[:], func=AF.Copy)
                ps = acc.tile([128, SC], F32, tag="acc")
                for t in range(NET):
                    nc.tensor.matmul(
                        ps[:],
                        lhsT=wk_sb[:, t, :],
                        rhs=xtk[:, t, :],
                        start=(t == 0),
                        stop=(t == NET - 1),
                    )
                nc.vector.tensor_copy(out=kT[:, ssl], in_=ps[:])
                ps = acc.tile([128, SC], F32, tag="acc")
                for t in range(NET):
                    nc.tensor.matmul(
                        ps[:],
                        lhsT=wv_sb[:, t, :],
                        rhs=xtv[:, t, :],
                        start=(t == 0),
                        stop=(t == NET - 1),
                    )
                vt = vtp.tile([128, SC], BF16, tag="vt")
                nc.scalar.activation(out=vt[:], in_=ps[:], func=AF.Copy)
                tp = trp.tile([128, 4, D], BF16, tag="tr")
                for i in range(4):
                    nc.tensor.transpose(
                        tp[:, i, :], vt[:, i * 128 : (i + 1) * 128], ident[:]
                    )
                nc.vector.tensor_copy(out=v_sb[:, c * 4 : (c + 1) * 4, :], in_=tp[:])

            def att(qc):
                qsl = slice(qc * SC, (qc + 1) * SC)
                nkt = 4 * (qc + 1)  # causal: k tiles 0..nkt-1
                work = [(h, kt) for h in range(G) for kt in range(nkt)]
                quads = [work[i : i + 4] for i in range(0, len(work), 4)]
                ptiles = {}

                def issue_s_quad(quad):
                    for h, kt in quad:
                        s_ps = acc.tile([128, SC], F32, tag="acc")
                        nc.tensor.matmul(
                            s_ps[:],
                            lhsT=kT[:, kt * 128 : (kt + 1) * 128],
                            rhs=qT[:, h, qsl],
                            start=True,
                            stop=True,
                        )
                        if kt >= nkt - 4:
                            nc.vector.tensor_add(
                                s_ps[:], s_ps[:], mask_sb[:, kt - 4 * qc, :]
                            )
                        p = pexp.tile([128, SC], BF16, tag="p")
                        nc.scalar.activation(out=p[:], in_=s_ps[:], func=AF.Exp)
                        ptiles[(h, kt)] = p

                issue_s_quad(quads[0])
                otile = {}
                ltile = {}
                for qi, quad in enumerate(quads):
                    h = quad[0][0]
                    if quad[0][1] == 0:
                        otile[h] = ops.tile([128, SC], F32, tag="o", name="o_ps")
                        ltile[h] = lps.tile([1, SC], F32, tag="l", name="l_ps")
                    if qi + 1 < len(quads):
                        issue_s_quad(quads[qi + 1])
                    # o-run then l-run: same-PSUM-bank matmuls back-to-back
                    for hh, kt in quad:
                        nc.tensor.matmul(
                            otile[hh][:],
                            lhsT=v_sb[:, kt, :],
                            rhs=ptiles[(hh, kt)][:],
                            start=(kt == 0),
                            stop=(kt == nkt - 1),
                        )
                    for hh, kt in quad:
                        nc.tensor.matmul(
                            ltile[hh][:],
                            lhsT=ones[:],
                            rhs=ptiles.pop((hh, kt))[:],
                            start=(kt == 0),
                            stop=(kt == nkt - 1),
                        )
                    if quad[-1][1] == nkt - 1:
                        rl = rlp.tile([1, SC], F32, tag="rl")
                        nc.vector.reciprocal_approx_fast(out=rl[:], in_=ltile[h][:])
                        rb = rbp.tile([128, SC], F32, tag="rb")
                        nc.gpsimd.partition_broadcast(rb[:], rl[:])
                        nc.vector.tensor_mul(onrm[:, h, qsl], otile[h][:], rb[:])

            def outproj(qc):
                for sti in range(4):
                    st = qc * 4 + sti
                    stl = slice(st * 128, (st + 1) * 128)
                    for ec in range(E // SC):
                        esl = slice(ec * SC, (ec + 1) * SC)
                        ps = acc.tile([128, SC], F32, tag="acc")
                        for h in range(G):
                            nc.tensor.matmul(
                                ps[:],
                                lhsT=onrm[:, h, stl],
                                rhs=wo_sb[:, h, esl],
                                start=(h == 0),
                                stop=(h == G - 1),
                            )
                        ob = obp.tile([128, SC], BF16, tag="ob")
                        g = sti * 4 + ec
                        if g % 2 == 0:
                            nc.vector.tensor_copy(out=ob[:], in_=ps[:])
                        else:
                            nc.scalar.activation(out=ob[:], in_=ps[:], func=AF.Copy)
                        if qc < 3:
                            eng = nc.sync if g % 2 == 0 else nc.gpsimd
                        else:
                            eng = nc.sync if g % 2 == 0 else nc.scalar
                        eng.dma_start(out=out[stl, esl], in_=ob[:])

            # ---- interleaved schedule; x prefetches positioned where
            # their pool-slot wait is already satisfied (a waiting DMA
            # blocks its issuing engine's whole instruction stream) ----
            proj(0)
            load_half(1, 2, xv, 0)
            load_half(1, 2, xv, 1)
            att(0)
            alloc_x(2)
            load_half(2, 0, xq, 0)
            load_half(2, 0, xq, 1)
            load_wo()
            proj(1)
            load_half(2, 1, xk, 0)
            load_half(2, 1, xk, 1)
            load_half(2, 2, xv, 0)
            load_half(2, 2, xv, 1)
            outproj(0)
            att(1)
            alloc_x(3)
            load_half(3, 0, xq, 0)
            load_half(3, 0, xq, 1)
            load_half(3, 1, xk, 0)
            load_half(3, 1, xk, 1)
            proj(2)
            load_half(3, 2, xv, 0)
            load_half(3, 2, xv, 1)
            outproj(1)
            att(2)
            proj(3)
            outproj(2)
            att(3)
            outproj(3)
    nc.compile()
    return nc


_NC_CACHE = None


def _get_nc():
    global _NC_CACHE
    if _NC_CACHE is None:
        _NC_CACHE = build_nc()
    return _NC_CACHE


def _tile_x(xT):
    # xT: [E, S] f32 -> [128, NSC, NET, SC] bf16 (p, chunk, etile, s2)
    return np.ascontiguousarray(
        xT.reshape(NET, 128, NSC, SC).transpose(1, 2, 0, 3)
    ).astype(NPBF)


def _prep_in_maps(query, key, value, attn_mask, Wq, Wk, Wv, Wo):
    query = np.asarray(query, dtype=np.float32)
    key = np.asarray(key, dtype=np.float32)
    value = np.asarray(value, dtype=np.float32)
    Wq = np.asarray(Wq, dtype=np.float32)
    Wk = np.asarray(Wk, dtype=np.float32)
    Wv = np.asarray(Wv, dtype=np.float32)
    Wo = np.asarray(Wo, dtype=np.float32)
    xqs = [_tile_x(query[b].T) for b in range(B)]
    xks = [_tile_x(key[b].T) for b in range(B)]
    xvs = [_tile_x(value[b].T) for b in range(B)]

    def tile_w(wT, width):
        # wT: [E, width] -> [128, NET, width]
        return np.ascontiguousarray(
            wT.reshape(NET, 128, width).transpose(1, 0, 2)
        ).astype(NPBF)

    in_maps = []
    for b in range(B):
        for g in range(HKV):
            wqT = Wq[g * GD : (g + 1) * GD, :].T * SCALE  # fold softmax scale
            wkT = Wk[g * D : (g + 1) * D, :].T
            wvT = Wv[g * D : (g + 1) * D, :].T
            woT = Wo[:, g * GD : (g + 1) * GD].T  # [GD, E]
            in_maps.append(
                {
                    "xq": xqs[b],
                    "xk": xks[b],
                    "xv": xvs[b],
                    "wq": tile_w(wqT, GD),
                    "wk": tile_w(wkT, D),
                    "wv": tile_w(wvT, D),
                    "wo": np.ascontiguousarray(
                        woT.reshape(G, 128, E).transpose(1, 0, 2)
                    ).astype(NPBF),
                }
            )
    return in_maps


def _run(inputs, trace=False, **kw):
    nc = _get_nc()
    in_maps = _prep_in_maps(**inputs)
    res = run_bass_kernel_spmd(nc, in_maps, list(range(NCORES)), trace=trace, **kw)
    outs = [np.asarray(r["out"]) for r in res.results]
    full = np.empty((B, S, E), dtype=np.float32)
    for b in range(B):
        acc = outs[b * HKV].astype(np.float32)
        for g in range(1, HKV):
            acc = acc + outs[b * HKV + g]
        full[b] = acc
    return full, res


def kernel(**inputs):
    full, _ = _run(inputs, trace=False)
    return full
